# revision 3
# baseline (speedup 1.0000x reference)
"""MultiHeadAttention (B=4, T=2048, d_model=1024, H=16, dh=64) on 8 trn2 cores.

Sharding: core c -> (batch b = c//2, head-group hg = c%2 of 8 heads).
Per-core kernel computes, feature-major throughout:
  QT = Wq_s^T x_q^T   [512, 2048]   (dq on partitions)
  KT = Wk_s^T x_k^T   [512, 2048]
  V  = x_v Wv_s       [2048, 512]   (s on partitions) + ones column per head
  ST = K_h Q_h^T      [s, t] per head; P = exp(ST/8 + mask_bias[s])
  ctxT_aug = V_aug^T P  -> [65, t]: rows 0..63 ctx^T, row 64 = softmax denom
  ctxT = ctxT_aug[0:64] * (1/denom)  broadcast over partitions via DRAM bounce
  outT_partial = Wo_s^T ctxT  [1024, 2048]
Host sums the two head-group partials per batch, transposes, adds bo.

x/weights/out travel as bf16 (halves DMA); matmuls accumulate fp32 in PSUM.
The timed variant (n_iters>1) runs the whole body in a For_i hardware loop,
2x-unrolled with parity-double-buffered QT/KT/V65 so iteration B's
projections interleave into iteration A's ACT(exp)-bound attention phase,
and iteration A's out-projection chains interleave into B's attention.
"""

import sys

sys.path.insert(0, "/opt/trn_rl_repo")

import numpy as np
import concourse.bass as bass
import concourse.tile as tile
from concourse import bacc, mybir
from concourse import bass_utils

B, T, DM = 4, 2048, 1024
H, DH = 16, 64
NHL = H // 2  # heads per core: 8
DQ = NHL * DH  # 512
KO = DM // 128  # 8 k-chunks over d_model
MQ = DQ // 128  # 4 M-tiles for q/k/v feature dim
NTB = T // 512  # 4 t-blocks
NS = T // 128  # 16 s-tiles
NMO = DM // 128  # 8 M-tiles for output dim
NTP = 2  # t halves for attention phase
F32 = mybir.dt.float32
F32R = mybir.dt.float32r
BF16 = mybir.dt.bfloat16
EXP = mybir.ActivationFunctionType.Exp
NPBF16 = mybir.dt.np(BF16)

_CACHE = {}


def _build_nc(n_iters=1, probe=None):
    nc = bacc.Bacc("TRN2", target_bir_lowering=False, num_devices=8)

    xqT = nc.declare_dram_parameter("xqT", [DM, T], BF16, isOutput=False)
    xkT = nc.declare_dram_parameter("xkT", [DM, T], BF16, isOutput=False)
    xvT = nc.declare_dram_parameter("xvT", [DM, T], BF16, isOutput=False)
    wq = nc.declare_dram_parameter("wq", [DM, DQ], BF16, isOutput=False)
    wk = nc.declare_dram_parameter("wk", [DM, DQ], BF16, isOutput=False)
    wv = nc.declare_dram_parameter("wv", [DM, DQ], BF16, isOutput=False)
    wo = nc.declare_dram_parameter("wo", [DQ, DM], BF16, isOutput=False)
    bqa = nc.declare_dram_parameter("bqa", [128, MQ], F32, isOutput=False)
    bka = nc.declare_dram_parameter("bka", [128, MQ], F32, isOutput=False)
    bv = nc.declare_dram_parameter("bv", [DQ], F32, isOutput=False)
    maskb = nc.declare_dram_parameter("maskb", [128, NS], F32, isOutput=False)
    outT = nc.declare_dram_parameter("outT", [DM, T], BF16, isOutput=True)

    den_dram = nc.dram_tensor("den_scratch", [2, NTP * NHL, 2 * 512], F32)

    if n_iters > 1 and n_iters % 4 == 0:
        unroll = 4
    elif n_iters > 1 and n_iters % 2 == 0:
        unroll = 2
    else:
        unroll = 1

    with tile.TileContext(nc) as tc:
        with (
            tc.tile_pool(name="consts", bufs=1) as consts,
            tc.tile_pool(name="big", bufs=1) as big,
            tc.tile_pool(name="wpool", bufs=1) as wpool,
            tc.tile_pool(name="xsl", bufs=2) as xsl_pool,
            tc.tile_pool(name="ppool", bufs=4) as ppool,
            tc.tile_pool(name="ctxp", bufs=3) as ctxp,
            tc.tile_pool(name="stage", bufs=2) as stage_pool,
            tc.tile_pool(name="recip", bufs=2) as recip_pool,
            tc.tile_pool(name="recipb", bufs=2) as recipb_pool,
            tc.tile_pool(name="ps_small", bufs=4, space="PSUM") as ps_small,
            tc.tile_pool(name="ps_st", bufs=2, space="PSUM") as ps_st,
        ):
            # ---- constants ----
            bqa_sb = consts.tile([128, MQ], F32, tag="bqa")
            bka_sb = consts.tile([128, MQ], F32, tag="bka")
            maskb_sb = consts.tile([128, NS], F32, tag="maskb")
            bvb_sb = consts.tile([128, DQ], F32, tag="bvb")
            ones_sb = consts.tile([128, NHL], F32, tag="ones")
            nc.vector.memset(ones_sb[:], 1.0)
            nc.gpsimd.dma_start(out=bqa_sb[:], in_=bqa[:, :])
            nc.gpsimd.dma_start(out=bka_sb[:], in_=bka[:, :])
            nc.gpsimd.dma_start(out=maskb_sb[:], in_=maskb[:, :])
            bv_ap = bv[:]
            nc.gpsimd.dma_start(
                out=bvb_sb[:],
                in_=bass.AP(tensor=bv_ap.tensor, offset=bv_ap.offset, ap=[[0, 128]] + list(bv_ap.ap)),
            )

            # ---- persistent tiles, parity-double-buffered (bf16) ----
            # 2 parities suffice for any unroll depth: iteration i+2's writes
            # land after iteration i's last reads (separated by a full
            # attention phase in the emission order).
            NPAR = min(unroll, 2)
            QT = {(p, m, tb): big.tile([128, 512], BF16, tag=f"QT_{p}_{m}_{tb}", name=f"QT_{p}_{m}_{tb}")
                  for p in range(NPAR) for m in range(MQ) for tb in range(NTB)}
            KT = {(p, m, tb): big.tile([128, 512], BF16, tag=f"KT_{p}_{m}_{tb}", name=f"KT_{p}_{m}_{tb}")
                  for p in range(NPAR) for m in range(MQ) for tb in range(NTB)}
            V65 = {(p, s): big.tile([128, NHL * 65], BF16, tag=f"V65_{p}_{s}", name=f"V65_{p}_{s}")
                   for p in range(NPAR) for s in range(NS)}
            wq_sb = wpool.tile([128, KO, DQ], BF16, tag="wq")
            wk_sb = wpool.tile([128, KO, DQ], BF16, tag="wk")
            wv_sb = wpool.tile([128, KO, DQ], BF16, tag="wv")
            wo_sb = {p: wpool.tile([128, MQ, DM], BF16, tag=f"wo_{p}", name=f"wo_{p}") for p in range(NPAR)}

            def dram_3d(param, col_off, ncols, row_stride):
                # [128 part, KO, ncols] view of DRAM param [DM, row_stride]
                ap0 = param[:, :]
                return bass.AP(
                    tensor=ap0.tensor,
                    offset=ap0.offset + col_off,
                    ap=[[row_stride, 128], [128 * row_stride, KO], [1, ncols]],
                )

            ctxTs = {}

            def p1_chunks(par):
                """13 closures: (q|k|v) projection per t-block + deferred wo load."""
                pp = par % NPAR
                chunks = []

                def load_qkv_weights():
                    for wdram, wsb in ((wq, wq_sb), (wk, wk_sb), (wv, wv_sb)):
                        nc.sync.dma_start(out=wsb[:], in_=dram_3d(wdram, 0, DQ, DQ))

                def load_wo():
                    # deferred to the end: wo_sb[pp] is still being read by
                    # iteration par-2's out-proj chains early in this window
                    wop = wo[:, :]
                    nc.sync.dma_start(
                        out=wo_sb[pp][:],
                        in_=bass.AP(tensor=wop.tensor, offset=wop.offset,
                                    ap=[[DM, 128], [128 * DM, MQ], [1, DM]]),
                    )

                def qk_chunk(name, xT, w_sb, b_sb, dst, tb):
                    def emit():
                        xt = xsl_pool.tile([128, KO, 512], BF16, tag="xsl")
                        nc.sync.dma_start(out=xt[:], in_=dram_3d(xT, tb * 512, 512, T))
                        for m in range(MQ):
                            psum = ps_small.tile([128, 512], F32, tag="small")
                            for ko in range(KO):
                                nc.tensor.matmul(
                                    psum[:],
                                    (w_sb[:, ko, m * 128 : (m + 1) * 128]),
                                    (xt[:, ko, :]),
                                    start=(ko == 0),
                                    stop=(ko == KO - 1),
                                )
                            nc.vector.tensor_scalar_add(dst[(pp, m, tb)][:], psum[:], b_sb[:, m : m + 1])
                    return emit

                def v_chunk(tb):
                    def emit():
                        xt = xsl_pool.tile([128, KO, 512], BF16, tag="xsl")
                        nc.sync.dma_start(out=xt[:], in_=dram_3d(xvT, tb * 512, 512, T))
                        for si in range(4):
                            s = tb * 4 + si
                            psum = ps_small.tile([128, 512], F32, tag="small")
                            for ko in range(KO):
                                nc.tensor.matmul(
                                    psum[:],
                                    (xt[:, ko, si * 128 : (si + 1) * 128]),
                                    (wv_sb[:, ko, :]),
                                    start=(ko == 0),
                                    stop=(ko == KO - 1),
                                )
                            v_view = V65[(pp, s)][:].rearrange("p (h c) -> p h c", c=65)
                            nc.vector.tensor_copy(
                                v_view[:, :, 64:65],
                                ones_sb[:].rearrange("p (h c) -> p h c", c=1),
                            )
                            nc.vector.tensor_add(
                                v_view[:, :, 0:64],
                                psum[:].rearrange("p (h c) -> p h c", c=64),
                                bvb_sb[:].rearrange("p (h c) -> p h c", c=64),
                            )
                    return emit

                first = True
                for tb in range(NTB):
                    for kind in ("q", "k", "v"):
                        if kind == "q":
                            fn = qk_chunk("q", xqT, wq_sb, bqa_sb, QT, tb)
                        elif kind == "k":
                            fn = qk_chunk("k", xkT, wk_sb, bka_sb, KT, tb)
                        else:
                            fn = v_chunk(tb)
                        if first:
                            def fn0(inner=fn):
                                load_qkv_weights()
                                inner()
                            chunks.append(fn0)
                            first = False
                        else:
                            chunks.append(fn)
                chunks.append(load_wo)
                return chunks

            def attn_head(par, tp, h):
                pp = par % NPAR
                q = h // 2
                hb = (h % 2) * 64
                ctxT_sb = ctxTs[(pp, tp)]
                ctx0 = ps_small.tile([65, 512], F32, tag="small", name=f"ctx0_{par}_{tp}_{h}")
                ctx1 = ps_small.tile([65, 512], F32, tag="small", name=f"ctx1_{par}_{tp}_{h}")
                ctxs = (ctx0, ctx1)
                p_tiles = {}

                def issue_st_exp(s):
                    st = ps_st.tile([128, 1024], F32, tag="st", name=f"st_{pp}_{tp}_{h}_{s}")
                    for tb2 in range(2):
                        nc.tensor.matmul(
                            st[:, tb2 * 512 : (tb2 + 1) * 512],
                            (KT[(pp, q, s // 4)][hb : hb + 64, (s % 4) * 128 : (s % 4 + 1) * 128]),
                            (QT[(pp, q, tp * 2 + tb2)][hb : hb + 64, :]),
                            start=True,
                            stop=True,
                        )
                    p_sb = ppool.tile([128, 1024], BF16, tag="p", name=f"p_{pp}_{tp}_{h}_{s}")
                    nc.scalar.activation(
                        out=p_sb[:],
                        in_=st[:],
                        func=EXP,
                        bias=maskb_sb[:, s : s + 1],
                        scale=0.125,
                    )
                    p_tiles[s] = p_sb

                def issue_ctx(s):
                    p_sb = p_tiles.pop(s)
                    for tb2 in range(2):
                        nc.tensor.matmul(
                            ctxs[tb2][:],
                            (V65[(pp, s)][:, h * 65 : (h + 1) * 65]),
                            (p_sb[:, tb2 * 512 : (tb2 + 1) * 512]),
                            start=(s == 0),
                            stop=(s == NS - 1),
                        )

                issue_st_exp(0)
                for s in range(1, NS):
                    issue_st_exp(s)
                    issue_ctx(s - 1)
                issue_ctx(NS - 1)

                # softmax denominator -> reciprocal -> partition-broadcast
                rc = recip_pool.tile([1, 2 * 512], F32, tag="recip", name=f"rc_{pp}_{tp}_{h}")
                for tb2 in range(2):
                    nc.vector.reciprocal(
                        out=rc[0:1, tb2 * 512 : (tb2 + 1) * 512],
                        in_=ctxs[tb2][64:65, :],
                    )
                rb = recipb_pool.tile([64, 2, 512], F32, tag="recipb", name=f"rb_{pp}_{tp}_{h}")
                nc.gpsimd.partition_broadcast(
                    rb[:].rearrange("p a b -> p (a b)"), rc[:], channels=64
                )
                for tb2 in range(2):
                    nc.vector.tensor_mul(
                        ctxT_sb[hb : hb + 64, q, tb2 * 512 : (tb2 + 1) * 512],
                        ctxs[tb2][0:64, :],
                        rb[:, tb2, :],
                    )

            def op_chunks(par):
                """32 closures: out-projection chain per (tp, mo, tb2)."""
                pp = par % NPAR
                chunks = []

                def op_chain(tp, mo, tb2):
                    def emit():
                        ctxT_sb = ctxTs[(pp, tp)]
                        psum = ps_small.tile([128, 512], F32, tag="small", name=f"op_{pp}_{tp}_{mo}_{tb2}")
                        for kq in range(MQ):
                            nc.tensor.matmul(
                                psum[:],
                                (wo_sb[pp][:, kq, mo * 128 : (mo + 1) * 128]),
                                (ctxT_sb[:, kq, tb2 * 512 : (tb2 + 1) * 512]),
                                start=(kq == 0),
                                stop=(kq == MQ - 1),
                            )
                        stg = stage_pool.tile([128, 512], BF16, tag="stage", name=f"stg_{pp}_{tp}_{mo}_{tb2}")
                        nc.vector.tensor_copy(stg[:], psum[:])
                        nc.sync.dma_start(
                            out=outT[
                                mo * 128 : (mo + 1) * 128,
                                tp * 1024 + tb2 * 512 : tp * 1024 + (tb2 + 1) * 512,
                            ],
                            in_=stg[:],
                        )
                    return emit

                for tp in range(NTP):
                    for mo in range(NMO):
                        for tb2 in range(2):
                            chunks.append(op_chain(tp, mo, tb2))
                return chunks

            def emit_attention(par, pending_ops, next_p1):
                slots = [(0, h) for h in range(NHL)] + [(1, h) for h in range(NHL)]
                pp = par % NPAR
                for tp, h in slots:
                    if h == 0:
                        ctxTs[(pp, tp)] = ctxp.tile(
                            [128, MQ, 1024], BF16, tag="ctxT", name=f"ctxT_{pp}_{tp}"
                        )
                    attn_head(par, tp, h)
                    for _ in range(2):
                        if pending_ops:
                            pending_ops.pop(0)()
                    if next_p1:
                        next_p1.pop(0)()

            def trip_body():
                for f in p1_chunks(0):
                    f()
                pending = []
                for par in range(unroll):
                    nxt = p1_chunks(par + 1) if par + 1 < unroll else []
                    emit_attention(par, pending, nxt)
                    pending = op_chunks(par)
                for f in pending:
                    f()

            if n_iters == 1:
                trip_body()
            elif probe == "flat":
                # sim-only: flatten the hardware loop so TimelineSim (no_exec)
                # can schedule it without resolving branches
                for _ in range(n_iters // unroll):
                    trip_body()
            else:
                with tc.For_i(0, n_iters // unroll, 1):
                    trip_body()

    nc.finalize()
    return nc


def _get_nc(n_iters=1, probe=None):
    key = f"nc{n_iters}_{probe}"
    if key not in _CACHE:
        _CACHE[key] = _build_nc(n_iters, probe=probe)
    return _CACHE[key]


def _get_runner(n_iters=1, donate=True, probe=None):
    """Persistent jitted 8-core runner (compiles the NEFF once per process)."""
    key = f"runner{n_iters}_{donate}_{probe}"
    if key in _CACHE:
        return _CACHE[key]
    import jax
    from jax.experimental.shard_map import shard_map
    from jax.sharding import Mesh, PartitionSpec
    from concourse import bass2jax

    nc = _get_nc(n_iters, probe=probe)
    bass2jax.install_neuronx_cc_hook()
    partition_name = nc.partition_id_tensor.name if nc.partition_id_tensor else None
    in_names, out_names, out_avals = [], [], []
    for alloc in nc.m.functions[0].allocations:
        if not isinstance(alloc, mybir.MemoryLocationSet):
            continue
        name = alloc.memorylocations[0].name
        if alloc.kind == "ExternalInput":
            if name != partition_name:
                in_names.append(name)
        elif alloc.kind == "ExternalOutput":
            out_names.append(name)
            out_avals.append(
                jax.core.ShapedArray(tuple(alloc.tensor_shape), mybir.dt.np(alloc.dtype))
            )
    n_params = len(in_names)
    n_outs = len(out_avals)
    all_names = list(in_names) + list(out_names)
    if partition_name is not None:
        all_names.append(partition_name)

    def _body(*args):
        operands = list(args)
        if partition_name is not None:
            operands.append(bass2jax.partition_id_tensor())
        outs = bass2jax._bass_exec_p.bind(
            *operands,
            out_avals=tuple(out_avals),
            in_names=tuple(all_names),
            out_names=tuple(out_names),
            lowering_input_output_aliases=(),
            sim_require_finite=True,
            sim_require_nnan=True,
            nc=nc,
        )
        return tuple(outs)

    devices = jax.devices()[:8]
    mesh = Mesh(np.asarray(devices), ("core",))
    in_specs = (PartitionSpec("core"),) * (n_params + n_outs)
    out_specs = (PartitionSpec("core"),) * n_outs
    jit_kwargs = dict(keep_unused=True)
    if donate:
        jit_kwargs["donate_argnums"] = tuple(range(n_params, n_params + n_outs))
    fn = jax.jit(
        shard_map(_body, mesh=mesh, in_specs=in_specs, out_specs=out_specs, check_rep=False),
        **jit_kwargs,
    )
    runner = (fn, list(in_names), list(out_names), list(out_avals))
    _CACHE[key] = runner
    return runner


def _run_cores(in_maps):
    """Run the SPMD kernel on 8 cores; returns list of per-core output dicts."""
    fn, in_names, out_names, out_avals = _get_runner()
    concat_in = [
        np.concatenate([np.asarray(m[name]) for m in in_maps], axis=0) for name in in_names
    ]
    concat_zeros = [
        np.zeros((8 * a.shape[0], *a.shape[1:]), a.dtype) for a in out_avals
    ]
    out_arrs = fn(*concat_in, *concat_zeros)
    return [
        {
            name: np.asarray(out_arrs[i]).reshape(8, *out_avals[i].shape)[c]
            for i, name in enumerate(out_names)
        }
        for c in range(8)
    ]


def kernel(
    x_Q, x_K, x_V, Wq, bq, Wk, bk, Wv, bv, Wo, bo, src_batch_lens, **_unused
):
    x_Q = np.ascontiguousarray(np.asarray(x_Q, dtype=np.float32))
    x_K = np.ascontiguousarray(np.asarray(x_K, dtype=np.float32))
    x_V = np.ascontiguousarray(np.asarray(x_V, dtype=np.float32))
    Wq = np.asarray(Wq, dtype=np.float32)
    Wk = np.asarray(Wk, dtype=np.float32)
    Wv = np.asarray(Wv, dtype=np.float32)
    Wo = np.asarray(Wo, dtype=np.float32)
    bq = np.asarray(bq, dtype=np.float32)
    bk = np.asarray(bk, dtype=np.float32)
    bv = np.asarray(bv, dtype=np.float32)
    bo = np.asarray(bo, dtype=np.float32)
    lens = np.asarray(src_batch_lens).astype(np.int64)

    in_maps = []
    s_idx = np.arange(T)
    for c in range(8):
        b, hg = c // 2, c % 2
        cols = slice(hg * DQ, (hg + 1) * DQ)
        mask_bias = np.where(s_idx < lens[b], 0.0, -1e9).astype(np.float32)
        in_maps.append(
            {
                "xqT": np.ascontiguousarray(x_Q[b].T).astype(NPBF16),
                "xkT": np.ascontiguousarray(x_K[b].T).astype(NPBF16),
                "xvT": np.ascontiguousarray(x_V[b].T).astype(NPBF16),
                "wq": np.ascontiguousarray(Wq[:, cols]).astype(NPBF16),
                "wk": np.ascontiguousarray(Wk[:, cols]).astype(NPBF16),
                "wv": np.ascontiguousarray(Wv[:, cols]).astype(NPBF16),
                "wo": np.ascontiguousarray(Wo[cols, :]).astype(NPBF16),
                "bqa": np.ascontiguousarray(bq[cols].reshape(MQ, 128).T),
                "bka": np.ascontiguousarray(bk[cols].reshape(MQ, 128).T),
                "bv": np.ascontiguousarray(bv[cols]),
                "maskb": np.ascontiguousarray(mask_bias.reshape(NS, 128).T),
            }
        )

    try:
        res = _run_cores(in_maps)
    except Exception:
        nc = _get_nc()
        res = bass_utils.run_bass_kernel_spmd(nc, in_maps, list(range(8))).results

    out = np.empty((B, T, DM), dtype=np.float32)
    for b in range(B):
        acc = res[2 * b]["outT"].astype(np.float32) + res[2 * b + 1]["outT"].astype(np.float32)
        out[b] = acc.T + bo[None, :]
    return out



# revision 6
# speedup vs baseline: 1.0281x; 1.0281x over previous
"""MultiHeadAttention (B=4, T=2048, d_model=1024, H=16, dh=64) on 8 trn2 cores.

Sharding: core c -> (batch b = c//2, head-group hg = c%2 of 8 heads).
Per-core kernel computes, feature-major throughout:
  QT = Wq_s^T x_q^T   [512, 2048]   (dq on partitions)
  KT = Wk_s^T x_k^T   [512, 2048]
  V  = x_v Wv_s       [2048, 512]   (s on partitions) + ones column per head
  ST = K_h Q_h^T      [s, t] per head; P = exp(ST/8 + mask_bias[s])
  ctxT_aug = V_aug^T P  -> [65, t]: rows 0..63 ctx^T, row 64 = softmax denom
  ctxT = ctxT_aug[0:64] * (1/denom)  broadcast over partitions via DRAM bounce
  outT_partial = Wo_s^T ctxT  [1024, 2048]
Host sums the two head-group partials per batch, transposes, adds bo.

x/weights/out travel as bf16 (halves DMA); matmuls accumulate fp32 in PSUM.
The timed variant (n_iters>1) runs the whole body in a For_i hardware loop,
2x-unrolled with parity-double-buffered QT/KT/V65 so iteration B's
projections interleave into iteration A's ACT(exp)-bound attention phase,
and iteration A's out-projection chains interleave into B's attention.
"""

import sys

sys.path.insert(0, "/opt/trn_rl_repo")

import numpy as np
import concourse.bass as bass
import concourse.tile as tile
from concourse import bacc, mybir
from concourse import bass_utils

B, T, DM = 4, 2048, 1024
H, DH = 16, 64
NHL = H // 2  # heads per core: 8
DQ = NHL * DH  # 512
KO = DM // 128  # 8 k-chunks over d_model
MQ = DQ // 128  # 4 M-tiles for q/k/v feature dim
NTB = T // 512  # 4 t-blocks
NS = T // 128  # 16 s-tiles
NMO = DM // 128  # 8 M-tiles for output dim
NTP = 2  # t halves for attention phase
F32 = mybir.dt.float32
F32R = mybir.dt.float32r
BF16 = mybir.dt.bfloat16
EXP = mybir.ActivationFunctionType.Exp
NPBF16 = mybir.dt.np(BF16)

_CACHE = {}


def _build_nc(n_iters=1, probe=None):
    nc = bacc.Bacc("TRN2", target_bir_lowering=False, num_devices=8)

    xqT = nc.declare_dram_parameter("xqT", [DM, T], BF16, isOutput=False)
    xkT = nc.declare_dram_parameter("xkT", [DM, T], BF16, isOutput=False)
    xvT = nc.declare_dram_parameter("xvT", [DM, T], BF16, isOutput=False)
    wq = nc.declare_dram_parameter("wq", [DM, DQ], BF16, isOutput=False)
    wk = nc.declare_dram_parameter("wk", [DM, DQ], BF16, isOutput=False)
    wv = nc.declare_dram_parameter("wv", [DM, DQ], BF16, isOutput=False)
    wo = nc.declare_dram_parameter("wo", [DQ, DM], BF16, isOutput=False)
    bqa = nc.declare_dram_parameter("bqa", [128, MQ], F32, isOutput=False)
    bka = nc.declare_dram_parameter("bka", [128, MQ], F32, isOutput=False)
    bv = nc.declare_dram_parameter("bv", [DQ], F32, isOutput=False)
    maskb = nc.declare_dram_parameter("maskb", [128, NS], F32, isOutput=False)
    outT = nc.declare_dram_parameter("outT", [DM, T], BF16, isOutput=True)

    den_dram = nc.dram_tensor("den_scratch", [2, NTP * NHL, 2 * 512], F32)

    if n_iters > 1 and n_iters % 4 == 0:
        unroll = 4
    elif n_iters > 1 and n_iters % 2 == 0:
        unroll = 2
    else:
        unroll = 1

    with tile.TileContext(nc) as tc:
        with (
            tc.tile_pool(name="consts", bufs=1) as consts,
            tc.tile_pool(name="big", bufs=1) as big,
            tc.tile_pool(name="wpool", bufs=1) as wpool,
            tc.tile_pool(name="xsl", bufs=2) as xsl_pool,
            tc.tile_pool(name="ppool", bufs=4) as ppool,
            tc.tile_pool(name="ctxp", bufs=3) as ctxp,
            tc.tile_pool(name="stage", bufs=2) as stage_pool,
            tc.tile_pool(name="recip", bufs=2) as recip_pool,
            tc.tile_pool(name="recipb", bufs=2) as recipb_pool,
            tc.tile_pool(name="ps_small", bufs=4, space="PSUM") as ps_small,
            tc.tile_pool(name="ps_st", bufs=2, space="PSUM") as ps_st,
        ):
            # ---- constants ----
            bqa_sb = consts.tile([128, MQ], F32, tag="bqa")
            bka_sb = consts.tile([128, MQ], F32, tag="bka")
            maskb_sb = consts.tile([128, NS], F32, tag="maskb")
            bvb_sb = consts.tile([128, DQ], F32, tag="bvb")
            ones_sb = consts.tile([128, NHL], F32, tag="ones")
            nc.vector.memset(ones_sb[:], 1.0)
            nc.gpsimd.dma_start(out=bqa_sb[:], in_=bqa[:, :])
            nc.gpsimd.dma_start(out=bka_sb[:], in_=bka[:, :])
            nc.gpsimd.dma_start(out=maskb_sb[:], in_=maskb[:, :])
            bv_ap = bv[:]
            nc.gpsimd.dma_start(
                out=bvb_sb[:],
                in_=bass.AP(tensor=bv_ap.tensor, offset=bv_ap.offset, ap=[[0, 128]] + list(bv_ap.ap)),
            )

            # ---- persistent tiles, parity-double-buffered (bf16) ----
            # 2 parities suffice for any unroll depth: iteration i+2's writes
            # land after iteration i's last reads (separated by a full
            # attention phase in the emission order).
            NPAR = min(unroll, 2)
            QT = {(p, m, tb): big.tile([128, 512], BF16, tag=f"QT_{p}_{m}_{tb}", name=f"QT_{p}_{m}_{tb}")
                  for p in range(NPAR) for m in range(MQ) for tb in range(NTB)}
            KT = {(p, m, tb): big.tile([128, 512], BF16, tag=f"KT_{p}_{m}_{tb}", name=f"KT_{p}_{m}_{tb}")
                  for p in range(NPAR) for m in range(MQ) for tb in range(NTB)}
            V65 = {(p, s): big.tile([128, NHL * 65], BF16, tag=f"V65_{p}_{s}", name=f"V65_{p}_{s}")
                   for p in range(NPAR) for s in range(NS)}
            wq_sb = wpool.tile([128, KO, DQ], BF16, tag="wq")
            wk_sb = wpool.tile([128, KO, DQ], BF16, tag="wk")
            wv_sb = wpool.tile([128, KO, DQ], BF16, tag="wv")
            wo_sb = {p: wpool.tile([128, MQ, DM], BF16, tag=f"wo_{p}", name=f"wo_{p}") for p in range(NPAR)}

            def dram_3d(param, col_off, ncols, row_stride):
                # [128 part, KO, ncols] view of DRAM param [DM, row_stride]
                ap0 = param[:, :]
                return bass.AP(
                    tensor=ap0.tensor,
                    offset=ap0.offset + col_off,
                    ap=[[row_stride, 128], [128 * row_stride, KO], [1, ncols]],
                )

            ctxTs = {}

            def p1_chunks(par):
                """13 closures: (q|k|v) projection per t-block + deferred wo load."""
                pp = par % NPAR
                chunks = []

                def load_qkv_weights():
                    for wdram, wsb in ((wq, wq_sb), (wk, wk_sb), (wv, wv_sb)):
                        nc.sync.dma_start(out=wsb[:], in_=dram_3d(wdram, 0, DQ, DQ))

                def load_wo():
                    # deferred to the end: wo_sb[pp] is still being read by
                    # iteration par-2's out-proj chains early in this window
                    wop = wo[:, :]
                    nc.sync.dma_start(
                        out=wo_sb[pp][:],
                        in_=bass.AP(tensor=wop.tensor, offset=wop.offset,
                                    ap=[[DM, 128], [128 * DM, MQ], [1, DM]]),
                    )

                def qk_chunk(name, xT, w_sb, b_sb, dst, tb):
                    def emit():
                        xt = xsl_pool.tile([128, KO, 512], BF16, tag="xsl")
                        nc.sync.dma_start(out=xt[:], in_=dram_3d(xT, tb * 512, 512, T))
                        for m in range(MQ):
                            psum = ps_small.tile([128, 512], F32, tag="small")
                            for ko in range(KO):
                                nc.tensor.matmul(
                                    psum[:],
                                    (w_sb[:, ko, m * 128 : (m + 1) * 128]),
                                    (xt[:, ko, :]),
                                    start=(ko == 0),
                                    stop=(ko == KO - 1),
                                )
                            nc.vector.tensor_scalar_add(dst[(pp, m, tb)][:], psum[:], b_sb[:, m : m + 1])
                    return emit

                def v_chunk(tb):
                    def emit():
                        xt = xsl_pool.tile([128, KO, 512], BF16, tag="xsl")
                        nc.sync.dma_start(out=xt[:], in_=dram_3d(xvT, tb * 512, 512, T))
                        for si in range(4):
                            s = tb * 4 + si
                            psum = ps_small.tile([128, 512], F32, tag="small")
                            for ko in range(KO):
                                nc.tensor.matmul(
                                    psum[:],
                                    (xt[:, ko, si * 128 : (si + 1) * 128]),
                                    (wv_sb[:, ko, :]),
                                    start=(ko == 0),
                                    stop=(ko == KO - 1),
                                )
                            v_view = V65[(pp, s)][:].rearrange("p (h c) -> p h c", c=65)
                            nc.vector.tensor_copy(
                                v_view[:, :, 64:65],
                                ones_sb[:].rearrange("p (h c) -> p h c", c=1),
                            )
                            nc.vector.tensor_add(
                                v_view[:, :, 0:64],
                                psum[:].rearrange("p (h c) -> p h c", c=64),
                                bvb_sb[:].rearrange("p (h c) -> p h c", c=64),
                            )
                    return emit

                first = True
                for tb in range(NTB):
                    for kind in ("q", "k", "v"):
                        if kind == "q":
                            fn = qk_chunk("q", xqT, wq_sb, bqa_sb, QT, tb)
                        elif kind == "k":
                            fn = qk_chunk("k", xkT, wk_sb, bka_sb, KT, tb)
                        else:
                            fn = v_chunk(tb)
                        if first:
                            def fn0(inner=fn):
                                load_qkv_weights()
                                inner()
                            chunks.append(fn0)
                            first = False
                        else:
                            chunks.append(fn)
                chunks.append(load_wo)
                return chunks

            def attn_head(par, tp, h):
                pp = par % NPAR
                q = h // 2
                hb = (h % 2) * 64
                ctxT_sb = ctxTs[(pp, tp)]
                ctx0 = ps_small.tile([65, 512], F32, tag="small", name=f"ctx0_{par}_{tp}_{h}")
                ctx1 = ps_small.tile([65, 512], F32, tag="small", name=f"ctx1_{par}_{tp}_{h}")
                ctxs = (ctx0, ctx1)
                p_tiles = {}

                def issue_st_exp(s):
                    st = ps_st.tile([128, 1024], F32, tag="st", name=f"st_{pp}_{tp}_{h}_{s}")
                    for tb2 in range(2):
                        nc.tensor.matmul(
                            st[:, tb2 * 512 : (tb2 + 1) * 512],
                            (KT[(pp, q, s // 4)][hb : hb + 64, (s % 4) * 128 : (s % 4 + 1) * 128]),
                            (QT[(pp, q, tp * 2 + tb2)][hb : hb + 64, :]),
                            start=True,
                            stop=True,
                        )
                    p_sb = ppool.tile([128, 1024], BF16, tag="p", name=f"p_{pp}_{tp}_{h}_{s}")
                    nc.scalar.activation(
                        out=p_sb[:],
                        in_=st[:],
                        func=EXP,
                        bias=maskb_sb[:, s : s + 1],
                        scale=0.125,
                    )
                    p_tiles[s] = p_sb

                def issue_ctx(s):
                    p_sb = p_tiles.pop(s)
                    for tb2 in range(2):
                        nc.tensor.matmul(
                            ctxs[tb2][:],
                            (V65[(pp, s)][:, h * 65 : (h + 1) * 65]),
                            (p_sb[:, tb2 * 512 : (tb2 + 1) * 512]),
                            start=(s == 0),
                            stop=(s == NS - 1),
                        )

                issue_st_exp(0)
                for s in range(1, NS):
                    issue_st_exp(s)
                    issue_ctx(s - 1)
                issue_ctx(NS - 1)

                # softmax denominator -> reciprocal -> partition-broadcast
                rc = recip_pool.tile([1, 2 * 512], F32, tag="recip", name=f"rc_{pp}_{tp}_{h}")
                for tb2 in range(2):
                    nc.vector.reciprocal(
                        out=rc[0:1, tb2 * 512 : (tb2 + 1) * 512],
                        in_=ctxs[tb2][64:65, :],
                    )
                idx = tp * NHL + h
                nc.gpsimd.dma_start(out=den_dram[pp, idx : idx + 1, :], in_=rc[:])
                rb = recipb_pool.tile([64, 2, 512], F32, tag="recipb", name=f"rb_{pp}_{tp}_{h}")
                dd = den_dram[pp, idx, :]
                nc.gpsimd.dma_start(
                    out=rb[:],
                    in_=bass.AP(
                        tensor=dd.tensor,
                        offset=dd.offset,
                        ap=[[0, 64], [512, 2], [1, 512]],
                    ),
                )
                for tb2 in range(2):
                    nc.vector.tensor_mul(
                        ctxT_sb[hb : hb + 64, q, tb2 * 512 : (tb2 + 1) * 512],
                        ctxs[tb2][0:64, :],
                        rb[:, tb2, :],
                    )

            def op_chunks(par):
                """32 closures: out-projection chain per (tp, mo, tb2)."""
                pp = par % NPAR
                chunks = []

                def op_chain(tp, mo, tb2):
                    def emit():
                        ctxT_sb = ctxTs[(pp, tp)]
                        psum = ps_small.tile([128, 512], F32, tag="small", name=f"op_{pp}_{tp}_{mo}_{tb2}")
                        for kq in range(MQ):
                            nc.tensor.matmul(
                                psum[:],
                                (wo_sb[pp][:, kq, mo * 128 : (mo + 1) * 128]),
                                (ctxT_sb[:, kq, tb2 * 512 : (tb2 + 1) * 512]),
                                start=(kq == 0),
                                stop=(kq == MQ - 1),
                            )
                        stg = stage_pool.tile([128, 512], BF16, tag="stage", name=f"stg_{pp}_{tp}_{mo}_{tb2}")
                        nc.vector.tensor_copy(stg[:], psum[:])
                        nc.sync.dma_start(
                            out=outT[
                                mo * 128 : (mo + 1) * 128,
                                tp * 1024 + tb2 * 512 : tp * 1024 + (tb2 + 1) * 512,
                            ],
                            in_=stg[:],
                        )
                    return emit

                for tp in range(NTP):
                    for mo in range(NMO):
                        for tb2 in range(2):
                            chunks.append(op_chain(tp, mo, tb2))
                return chunks

            def emit_attention(par, pending_ops, next_p1):
                slots = [(0, h) for h in range(NHL)] + [(1, h) for h in range(NHL)]
                pp = par % NPAR
                for tp, h in slots:
                    if h == 0:
                        ctxTs[(pp, tp)] = ctxp.tile(
                            [128, MQ, 1024], BF16, tag="ctxT", name=f"ctxT_{pp}_{tp}"
                        )
                    attn_head(par, tp, h)
                    for _ in range(2):
                        if pending_ops:
                            pending_ops.pop(0)()
                    if next_p1:
                        next_p1.pop(0)()

            def trip_body():
                for f in p1_chunks(0):
                    f()
                pending = []
                for par in range(unroll):
                    nxt = p1_chunks(par + 1) if par + 1 < unroll else []
                    emit_attention(par, pending, nxt)
                    pending = op_chunks(par)
                for f in pending:
                    f()

            if n_iters == 1:
                trip_body()
            elif probe == "flat":
                # sim-only: flatten the hardware loop so TimelineSim (no_exec)
                # can schedule it without resolving branches
                for _ in range(n_iters // unroll):
                    trip_body()
            else:
                with tc.For_i(0, n_iters // unroll, 1):
                    trip_body()

    nc.finalize()
    return nc


def _get_nc(n_iters=1, probe=None):
    key = f"nc{n_iters}_{probe}"
    if key not in _CACHE:
        _CACHE[key] = _build_nc(n_iters, probe=probe)
    return _CACHE[key]


def _get_runner(n_iters=1, donate=True, probe=None):
    """Persistent jitted 8-core runner (compiles the NEFF once per process)."""
    key = f"runner{n_iters}_{donate}_{probe}"
    if key in _CACHE:
        return _CACHE[key]
    import jax
    from jax.experimental.shard_map import shard_map
    from jax.sharding import Mesh, PartitionSpec
    from concourse import bass2jax

    nc = _get_nc(n_iters, probe=probe)
    bass2jax.install_neuronx_cc_hook()
    partition_name = nc.partition_id_tensor.name if nc.partition_id_tensor else None
    in_names, out_names, out_avals = [], [], []
    for alloc in nc.m.functions[0].allocations:
        if not isinstance(alloc, mybir.MemoryLocationSet):
            continue
        name = alloc.memorylocations[0].name
        if alloc.kind == "ExternalInput":
            if name != partition_name:
                in_names.append(name)
        elif alloc.kind == "ExternalOutput":
            out_names.append(name)
            out_avals.append(
                jax.core.ShapedArray(tuple(alloc.tensor_shape), mybir.dt.np(alloc.dtype))
            )
    n_params = len(in_names)
    n_outs = len(out_avals)
    all_names = list(in_names) + list(out_names)
    if partition_name is not None:
        all_names.append(partition_name)

    def _body(*args):
        operands = list(args)
        if partition_name is not None:
            operands.append(bass2jax.partition_id_tensor())
        outs = bass2jax._bass_exec_p.bind(
            *operands,
            out_avals=tuple(out_avals),
            in_names=tuple(all_names),
            out_names=tuple(out_names),
            lowering_input_output_aliases=(),
            sim_require_finite=True,
            sim_require_nnan=True,
            nc=nc,
        )
        return tuple(outs)

    devices = jax.devices()[:8]
    mesh = Mesh(np.asarray(devices), ("core",))
    in_specs = (PartitionSpec("core"),) * (n_params + n_outs)
    out_specs = (PartitionSpec("core"),) * n_outs
    jit_kwargs = dict(keep_unused=True)
    if donate:
        jit_kwargs["donate_argnums"] = tuple(range(n_params, n_params + n_outs))
    fn = jax.jit(
        shard_map(_body, mesh=mesh, in_specs=in_specs, out_specs=out_specs, check_rep=False),
        **jit_kwargs,
    )
    runner = (fn, list(in_names), list(out_names), list(out_avals))
    _CACHE[key] = runner
    return runner


def _run_cores(in_maps):
    """Run the SPMD kernel on 8 cores; returns list of per-core output dicts."""
    fn, in_names, out_names, out_avals = _get_runner()
    concat_in = [
        np.concatenate([np.asarray(m[name]) for m in in_maps], axis=0) for name in in_names
    ]
    concat_zeros = [
        np.zeros((8 * a.shape[0], *a.shape[1:]), a.dtype) for a in out_avals
    ]
    out_arrs = fn(*concat_in, *concat_zeros)
    return [
        {
            name: np.asarray(out_arrs[i]).reshape(8, *out_avals[i].shape)[c]
            for i, name in enumerate(out_names)
        }
        for c in range(8)
    ]


def kernel(
    x_Q, x_K, x_V, Wq, bq, Wk, bk, Wv, bv, Wo, bo, src_batch_lens, **_unused
):
    x_Q = np.ascontiguousarray(np.asarray(x_Q, dtype=np.float32))
    x_K = np.ascontiguousarray(np.asarray(x_K, dtype=np.float32))
    x_V = np.ascontiguousarray(np.asarray(x_V, dtype=np.float32))
    Wq = np.asarray(Wq, dtype=np.float32)
    Wk = np.asarray(Wk, dtype=np.float32)
    Wv = np.asarray(Wv, dtype=np.float32)
    Wo = np.asarray(Wo, dtype=np.float32)
    bq = np.asarray(bq, dtype=np.float32)
    bk = np.asarray(bk, dtype=np.float32)
    bv = np.asarray(bv, dtype=np.float32)
    bo = np.asarray(bo, dtype=np.float32)
    lens = np.asarray(src_batch_lens).astype(np.int64)

    in_maps = []
    s_idx = np.arange(T)
    for c in range(8):
        b, hg = c // 2, c % 2
        cols = slice(hg * DQ, (hg + 1) * DQ)
        mask_bias = np.where(s_idx < lens[b], 0.0, -1e9).astype(np.float32)
        in_maps.append(
            {
                "xqT": np.ascontiguousarray(x_Q[b].T).astype(NPBF16),
                "xkT": np.ascontiguousarray(x_K[b].T).astype(NPBF16),
                "xvT": np.ascontiguousarray(x_V[b].T).astype(NPBF16),
                "wq": np.ascontiguousarray(Wq[:, cols]).astype(NPBF16),
                "wk": np.ascontiguousarray(Wk[:, cols]).astype(NPBF16),
                "wv": np.ascontiguousarray(Wv[:, cols]).astype(NPBF16),
                "wo": np.ascontiguousarray(Wo[cols, :]).astype(NPBF16),
                "bqa": np.ascontiguousarray(bq[cols].reshape(MQ, 128).T),
                "bka": np.ascontiguousarray(bk[cols].reshape(MQ, 128).T),
                "bv": np.ascontiguousarray(bv[cols]),
                "maskb": np.ascontiguousarray(mask_bias.reshape(NS, 128).T),
            }
        )

    try:
        res = _run_cores(in_maps)
    except Exception:
        nc = _get_nc()
        res = bass_utils.run_bass_kernel_spmd(nc, in_maps, list(range(8))).results

    out = np.empty((B, T, DM), dtype=np.float32)
    for b in range(B):
        acc = res[2 * b]["outT"].astype(np.float32) + res[2 * b + 1]["outT"].astype(np.float32)
        out[b] = acc.T + bo[None, :]
    return out

